# revision 6
# baseline (speedup 1.0000x reference)
"""Trainium2 Bass kernel for nn_MultiHeadAttention_59992103190912.

Strategy (8 cores): data-parallel over batch (4) x tensor-parallel over heads
(2-way, 8 heads/core).  Each core computes, for its (batch b, head-half hh):

    LN folded into weights:  xn^T = [xs; c; 1] with xs = x^T*rstd, c = -mu*rstd
    K^T, Q^T = W_aug^T @ [xs; c; 1]          [512 feat, T]  (bf16)
    V        = [xs; c; 1]^T @ Wv_aug         [T, 65*HL]     (bf16, interleaved
                                              ones column per head -> row 64 of
                                              the PV accumulator = softmax den)
    S^T      = K^T_h k-tiles x Q^T_h         [k, q] fp32 PSUM
    P        = exp(SCALE * S^T .* mask^T)    via 3 balanced routes:
                 A: ACT exp(SCALE*s) psum->bf16, Pool pow(u, mask)
                 B: DVE mult(s, mask) psum->f32, ACT exp(SCALE*.)->bf16
                 C: DVE mult(s, mask),          Pool pow(e^SCALE, msm)->bf16
    OT_u     = V_ext^T-chunks @ P            [65, q] PSUM; row 64 = den
    avT      = OT_u[0:64] * bcast(1/den)     DVE recip + PE bcast matmul
    out_p    = avT^T @ Wo_half               [T, D] partial (host sums + bo)

Mask is multiplicative on the scaled scores; exp(s*m) == exp(s)^m lets the
PSUM drain (ACT-only or DVE-only) and the mask application (Pool-capable)
be assigned to whichever engine has slack.
"""

import sys

sys.path.insert(0, "/opt/trn_rl_repo")

from contextlib import ExitStack
from dataclasses import dataclass

import numpy as np
import ml_dtypes

import concourse.bass as bass
import concourse.tile as tile
from concourse import mybir

F32 = mybir.dt.float32
F32R = mybir.dt.float32r
BF16 = mybir.dt.bfloat16
F16 = mybir.dt.float16
I16 = mybir.dt.int16
AF = mybir.ActivationFunctionType
ALU = mybir.AluOpType
NPBF16 = ml_dtypes.bfloat16

# Schraudolph exp-as-bitcast constants (bf16 target): for x = scaled masked
# score, bits16(exp(x)) ~= int16(x * 2^7/ln2 + (127*2^7 + C)); the int16
# bit pattern IS the bf16 value.  C tunes the trunc/sawtooth bias.
EXP_A = 2.0 ** 7 / float(np.log(2.0))   # 184.6650
EXP_B = 127.0 * 2 ** 7 - 10.0


@dataclass(frozen=True)
class Dims:
    T: int = 2048      # sequence length
    D: int = 1024      # model dim
    HL: int = 8        # heads per core
    DH: int = 64       # head dim
    QC: int = 512      # q-chunk width for attention phase
    LN_EPS: float = 1e-5

    @property
    def F(self):       # features per core (= HL * DH)
        return self.HL * self.DH

    @property
    def ND(self):      # d-chunks of 128
        return self.D // 128

    @property
    def NF(self):      # feature tiles of 128
        return self.F // 128

    @property
    def NKT(self):     # k tiles of 128
        return self.T // 128

    @property
    def NQC(self):     # q chunks
        return self.T // self.QC

    @property
    def NTC(self):     # 512-wide t-chunks (stats/QKV phase)
        return self.T // 512

    @property
    def SCALE(self):
        return self.DH ** -0.5


# Mask+exp routing: gpsimd pow is software-emulated (~165us per tile on
# HW), and Pool cannot read PSUM, so every score tile takes the same path:
# DVE multiplies the PSUM scores by the mask, ACT applies exp.


def build_bass(d: Dims = Dims(), dbg: str | None = None) -> bass.Bass:
    assert d.T % 512 == 0 and d.D % 128 == 0 and d.F % 128 == 0
    assert d.QC == 512 and d.T % d.QC == 0
    nc = bass.Bass()

    xT_d = nc.declare_dram_parameter("xT", [d.D, d.T], BF16, isOutput=False)
    mk_d = nc.declare_dram_parameter("maskT", [d.T, d.T], BF16, isOutput=False)
    wq_d = nc.declare_dram_parameter("Wq", [d.D + 2, d.F], BF16, isOutput=False)
    wk_d = nc.declare_dram_parameter("Wk", [d.D + 2, d.F], BF16, isOutput=False)
    wv_d = nc.declare_dram_parameter("Wv", [d.D + 2, d.HL * 65], BF16,
                                     isOutput=False)
    wo_d = nc.declare_dram_parameter("Wo", [d.F, d.D], BF16, isOutput=False)
    out_d = nc.declare_dram_parameter("out_p", [d.T, d.D], F32, isOutput=True)
    dump_d = None
    if dbg is not None:
        dump_d = nc.declare_dram_parameter("dump", [2048, 2048], F32,
                                           isOutput=True)

    with tile.TileContext(nc) as tc, \
            nc.allow_low_precision(reason="bf16 data path; 2e-2 rel-err gate"):
        with ExitStack() as ctx:
            _body(ctx, tc, d, xT_d, mk_d, wq_d, wk_d, wv_d, wo_d, out_d,
                  dbg=dbg, dump_d=dump_d)
    return nc


def _split_multi_waits(nc: bass.Bass, maxw: int = 1):
    """Walrus codegen rejects instructions with more than one sync-wait
    ("Too many sync wait commands", CoreV3GenImpl setupSyncWait).  The Tile
    kernel-tail drain accumulates one wait per outstanding logical proc.
    Split any such instruction: hoist the extra waits onto same-engine no-op
    instructions inserted immediately before it (waits are AND conditions,
    so waiting sequentially is equivalent)."""
    uid = 0
    for fn in nc.m.functions:
        for bb in fn.blocks:
            insts = bb.instructions
            i = 0
            while i < len(insts):
                inst = insts[i]
                si = inst.sync_info
                if si is not None and len(si.on_wait) > maxw:
                    waits = list(si.on_wait)
                    head, keep = waits[:-maxw], waits[-maxw:]
                    for j, w in enumerate(head):
                        nop = mybir.InstNoOp(
                            name=f"bass_splitw_{uid}", engine=inst.engine,
                            ins=[], outs=[], bass_nofuse=True,
                        )
                        uid += 1
                        nop.sync_info = mybir.SyncInfo(on_wait=[w],
                                                       on_update=[])
                        insts.insert(i, nop)
                        i += 1
                    inst.sync_info = mybir.SyncInfo(on_wait=keep,
                                                    on_update=list(si.on_update))
                i += 1


def _body(ctx, tc, d: Dims, xT_d, mk_d, wq_d, wk_d, wv_d, wo_d, out_d,
          dbg=None, dump_d=None):
    nc = tc.nc
    ts = bass.ts

    persist = ctx.enter_context(tc.tile_pool(name="persist", bufs=1))
    ones_f = persist.tile([128, 128], F32)
    nc.vector.memset(ones_f, 1.0)
    # f32r / bf16 "ones" stationary vectors (memset cannot emit f32r)
    ones_1x128_r = persist.tile([1, 128], F32R)
    nc.vector.tensor_copy(ones_1x128_r, ones_f[0:1, 0:128])
    ones_128x1_b = persist.tile([128, 1], BF16)
    nc.vector.tensor_copy(ones_128x1_b, ones_f[:, 0:1])
    ones_1x64_r = persist.tile([1, 64], F32R)
    nc.vector.tensor_copy(ones_1x64_r, ones_f[0:1, 0:64])

    # mask pool lives early so q-chunk 0 tiles can prefetch during QKV;
    # sized for two q-chunks so the next chunk's prefetch DMAs (in-order on
    # the SP queue, ahead of proj output writes) never block on slots
    mk_p = ctx.enter_context(tc.tile_pool(name="mk", bufs=d.NKT))

    # aug rows + xs survive into phase 2 (s12 closes them before phase 3)
    s12 = ctx.enter_context(ExitStack())
    aug_p = s12.enter_context(tc.tile_pool(name="aug", bufs=1, side="right"))
    aug_f = aug_p.tile([2, d.T], F32)  # row 0: c = -mu*rstd, row 1: ones
    nc.vector.memset(aug_f, 1.0)
    aug = aug_p.tile([2, d.T], BF16)
    xs_p = s12.enter_context(tc.tile_pool(name="xs", bufs=d.ND, side="right"))
    xs = [xs_p.tile([128, d.T], BF16, tag="xs", name=f"xs{i}")
          for i in range(d.ND)]

    # =========================================================
    # Phase 1: LN stats per 512-wide t-chunk (pipelined with DMA)
    # =========================================================
    with ExitStack() as s1:
        xt_p = s1.enter_context(tc.tile_pool(name="xt", bufs=2 * d.ND))
        sq_p = s1.enter_context(tc.tile_pool(name="sq", bufs=2))
        rows1 = s1.enter_context(tc.tile_pool(name="rows1", bufs=2 * d.NTC))
        rb1_p = s1.enter_context(tc.tile_pool(name="rb1", bufs=d.NTC))
        ps_st = s1.enter_context(tc.tile_pool(name="ps_st", bufs=4,
                                              space="PSUM"))
        ps_rb = s1.enter_context(tc.tile_pool(name="ps_rb", bufs=2,
                                              space="PSUM"))

        for tci in range(d.NTC):
            xts = []
            for dt in range(d.ND):
                xt = xt_p.tile([128, 512], BF16, tag="xt",
                               name=f"xt{tci}_{dt}")
                nc.sync.dma_start(out=xt,
                                  in_=xT_d[ts(dt, 128), ts(tci, 512)])
                xts.append(xt)
            sum_ps = ps_st.tile([1, 512], F32, tag="pss", name=f"sum{tci}")
            msq_ps = ps_st.tile([1, 512], F32, tag="pss", name=f"msq{tci}")
            for dt in range(d.ND):
                sq = sq_p.tile([128, 512], BF16, tag="sq",
                               name=f"sq{tci}_{dt}")
                nc.vector.tensor_mul(sq, xts[dt], xts[dt])
                nc.tensor.matmul(sum_ps, ones_128x1_b, xts[dt],
                                 start=(dt == 0), stop=(dt == d.ND - 1))
                nc.tensor.matmul(msq_ps, ones_128x1_b, sq,
                                 start=(dt == 0), stop=(dt == d.ND - 1))
            mu = rows1.tile([1, 512], F32, tag="r1", name=f"mu{tci}")
            m2 = rows1.tile([1, 512], F32, tag="r1", name=f"m2{tci}")
            nc.scalar.activation(mu, sum_ps, AF.Copy, scale=1.0 / d.D)
            nc.scalar.activation(m2, msq_ps, AF.Copy, scale=1.0 / d.D)
            mu2 = rows1.tile([1, 512], F32, tag="r1", name=f"mu2{tci}")
            nc.vector.tensor_mul(mu2, mu, mu)
            var = rows1.tile([1, 512], F32, tag="r1", name=f"var{tci}")
            nc.vector.scalar_tensor_tensor(var, m2, d.LN_EPS, mu2,
                                           ALU.add, ALU.subtract)
            lnv = rows1.tile([1, 512], F32, tag="r1", name=f"lnv{tci}")
            nc.scalar.activation(lnv, var, AF.Ln)
            rstd = rows1.tile([1, 512], F32R, tag="r1", name=f"rstd{tci}")
            nc.scalar.activation(rstd, lnv, AF.Exp, scale=-0.5)
            # c = -mu*rstd into aug row 0; bf16 copy of both aug rows
            nc.vector.scalar_tensor_tensor(
                aug_f[0:1, ts(tci, 512)], mu, -1.0, rstd.bitcast(F32),
                ALU.mult, ALU.mult)
            nc.vector.tensor_copy(aug[:, ts(tci, 512)],
                                  aug_f[:, ts(tci, 512)])
            # broadcast rstd over 128 partitions -> bf16, xs = xt * rstd
            rb_ps = ps_rb.tile([128, 512], F32, tag="rb", name=f"rbp{tci}")
            nc.tensor.matmul(rb_ps, ones_1x128_r, rstd, start=True, stop=True)
            rb_sb = rb1_p.tile([128, 512], BF16, tag="rb1", name=f"rb{tci}")
            nc.scalar.copy(rb_sb, rb_ps)
            for dt in range(d.ND):
                nc.vector.tensor_mul(xs[dt][:, ts(tci, 512)], xts[dt], rb_sb)

    if dbg == "xs":
        xf = aug_p.tile([128, d.T], F32)
        for dt in range(d.ND):
            nc.vector.tensor_copy(xf, xs[dt])
            nc.sync.dma_start(out=dump_d[ts(dt, 128), 0:d.T], in_=xf)
        af = aug_p.tile([2, d.T], F32)
        nc.vector.tensor_copy(af, aug)
        nc.sync.dma_start(out=dump_d[d.D + 2:d.D + 4, 0:d.T], in_=af)
        return

    # prefetch first q-chunk's mask tiles (bf16) before the QKV phase
    mk_tiles = {}

    def load_mask(qc, j):
        t = mk_p.tile([128, 2 * d.QC], BF16, tag="mk", name=f"mk{qc}_{j}")
        nc.sync.dma_start(out=t[:, 0:d.QC],
                          in_=mk_d[ts(2 * j, 128), ts(qc, d.QC)])
        nc.sync.dma_start(out=t[:, d.QC:2 * d.QC],
                          in_=mk_d[ts(2 * j + 1, 128), ts(qc, d.QC)])
        mk_tiles[(qc, j)] = t

    for j in range(d.NKT // 2):
        load_mask(0, j)

    # =========================================================
    # Phase 2: projections, order K -> Q -> V (attention starts on K+Q)
    # =========================================================
    qt_p = ctx.enter_context(tc.tile_pool(name="qt", bufs=d.NF))
    ktl_p = ctx.enter_context(tc.tile_pool(name="ktl", bufs=d.NF))
    vx_p = ctx.enter_context(tc.tile_pool(name="vx", bufs=d.NKT))
    qt = [qt_p.tile([128, d.T], BF16, tag="qt", name=f"qt{i}")
          for i in range(d.NF)]
    kt = [ktl_p.tile([128, d.T], BF16, tag="ktl", name=f"ktt{i}")
          for i in range(d.NF)]
    vx = [vx_p.tile([128, d.HL * 65], BF16, tag="vx", name=f"vx{i}")
          for i in range(d.NKT)]

    # wq stays resident: Q projections for q-chunks 1+ are interleaved into
    # the attention phase (PE slack there), so its pool must outlive phase 2
    wq_p = ctx.enter_context(tc.tile_pool(name="wqp", bufs=d.ND))
    wqt_p = ctx.enter_context(tc.tile_pool(name="wqt", bufs=1))
    wq_main = []
    wq_tail = None

    def emit_qproj(tci):
        for ft in range(d.NF):
            ps = ps_b.tile([128, 512], F32, tag="psb",
                           name=f"wqps{ft}_{tci}")
            for dc in range(d.ND):
                nc.tensor.matmul(
                    ps, wq_main[dc][:, ts(ft, 128)],
                    xs[dc][:, ts(tci, 512)],
                    start=(dc == 0), stop=False,
                )
            nc.tensor.matmul(
                ps, wq_tail[:, ts(ft, 128)], aug[:, ts(tci, 512)],
                start=False, stop=True,
            )
            nc.scalar.copy(qt[ft][:, ts(tci, 512)], ps)

    with ExitStack() as s2:
        w_p = s2.enter_context(tc.tile_pool(name="w", bufs=d.ND + 2))
        wt_p = s2.enter_context(tc.tile_pool(name="wt", bufs=2))
        ps_qk = s2.enter_context(tc.tile_pool(name="ps_qk", bufs=3,
                                              space="PSUM"))
        ps_v = s2.enter_context(tc.tile_pool(name="ps_v", bufs=2,
                                             space="PSUM"))

        def load_w(wd, pfx, width, main_p=None, tail_p=None):
            wmain = []
            for dc in range(d.ND):
                t = (main_p or w_p).tile([128, width], BF16, tag="w",
                                         name=f"{pfx}{dc}")
                nc.sync.dma_start(out=t, in_=wd[ts(dc, 128), :])
                wmain.append(t)
            wtail = (tail_p or wt_p).tile([2, width], BF16, tag="wt",
                                          name=f"{pfx}t")
            nc.sync.dma_start(out=wtail, in_=wd[d.D:d.D + 2, :])
            return wmain, wtail

        # ---- K fully (transposed layout [feat, t])
        wmain, wtail = load_w(wk_d, "wk", d.F)
        for ft in range(d.NF):
            for tci in range(d.NTC):
                ps = ps_qk.tile([128, 512], F32, tag="psq",
                                name=f"wkps{ft}_{tci}")
                for dc in range(d.ND):
                    nc.tensor.matmul(
                        ps, wmain[dc][:, ts(ft, 128)],
                        xs[dc][:, ts(tci, 512)],
                        start=(dc == 0), stop=False,
                    )
                nc.tensor.matmul(
                    ps, wtail[:, ts(ft, 128)], aug[:, ts(tci, 512)],
                    start=False, stop=True,
                )
                nc.scalar.copy(kt[ft][:, ts(tci, 512)], ps)

        # ---- Q for the first q-chunk only (rest interleaves into phase 3)
        wq_main, wq_tail = load_w(wq_d, "wq", d.F, main_p=wq_p, tail_p=wqt_p)
        for ft in range(d.NF):
            ps = ps_qk.tile([128, 512], F32, tag="psq", name=f"wqp0_{ft}")
            for dc in range(d.ND):
                nc.tensor.matmul(
                    ps, wq_main[dc][:, ts(ft, 128)], xs[dc][:, 0:512],
                    start=(dc == 0), stop=False,
                )
            nc.tensor.matmul(ps, wq_tail[:, ts(ft, 128)], aug[:, 0:512],
                             start=False, stop=True)
            nc.scalar.copy(qt[ft][:, 0:512], ps)

        # ---- V: host-interleaved [64 cols | ones col] per head.  Matmul N
        # is capped at 512 elements and an accumulation group must stay in
        # one PSUM bank, so emit per 4-head group (N=260, own PSUM tile).
        vmain, vtail = load_w(wv_d, "wv", d.HL * 65)
        vgw = min(4, d.HL) * 65  # 4-head group width
        ngr = (d.HL * 65) // vgw
        for tt in range(d.NKT):
            for g in range(ngr):
                sl = slice(g * vgw, (g + 1) * vgw)
                v_ps = ps_v.tile([128, vgw], F32, tag="psv",
                                 name=f"vps{tt}_{g}")
                for dc in range(d.ND):
                    nc.tensor.matmul(
                        v_ps, xs[dc][:, ts(tt, 128)], vmain[dc][:, sl],
                        start=(dc == 0), stop=False,
                    )
                nc.tensor.matmul(v_ps, aug[:, ts(tt, 128)],
                                 vtail[:, sl], start=False, stop=True)
                nc.scalar.copy(vx[tt][:, sl], v_ps)

    if dbg == "qkv":
        tmp_p = ctx.enter_context(tc.tile_pool(name="dbgt", bufs=1))
        tf = tmp_p.tile([128, d.T], F32)
        for ft in range(d.NF):
            nc.vector.tensor_copy(tf, qt[ft])
            nc.sync.dma_start(out=dump_d[ts(ft, 128), 0:d.T], in_=tf)
            nc.vector.tensor_copy(tf, kt[ft])
            nc.sync.dma_start(out=dump_d[d.F + ft * 128:d.F + (ft + 1) * 128,
                                         0:d.T], in_=tf)
        for tt in range(d.NKT):
            nc.vector.tensor_copy(tf[:, 0:d.HL * 65], vx[tt])
            nc.sync.dma_start(
                out=dump_d[2 * d.F + tt * 128:2 * d.F + (tt + 1) * 128,
                           0:d.HL * 65], in_=tf[:, 0:d.HL * 65])
        return

    # =========================================================
    # Phase 3: attention + output projection, per q-chunk
    # =========================================================
    # (xs/aug stay resident: the interleaved Q projections still read them)
    wo_p = ctx.enter_context(tc.tile_pool(name="wo", bufs=d.NF))
    stg_p = ctx.enter_context(tc.tile_pool(name="stg", bufs=3))   # f32 msm
    stg16_p = ctx.enter_context(tc.tile_pool(name="stg16", bufs=4))  # f16
    # P tiles: at pair j of head h, p(h-1, j..7) await PV while p(h, 0..j)
    # have been produced -> ~9-11 live
    pb_p = ctx.enter_context(tc.tile_pool(name="pb", bufs=12))    # bf16 P
    avt_p = ctx.enter_context(tc.tile_pool(name="avt", bufs=2 * d.NF + 1))
    ob_p = ctx.enter_context(tc.tile_pool(name="ob", bufs=2))
    rows3 = ctx.enter_context(tc.tile_pool(name="rows3", bufs=2))
    rbe_p = ctx.enter_context(tc.tile_pool(name="rbe", bufs=2))
    # st slots need only depth 2: PE alternates st/pv pairs (1.7us cadence)
    # while the DVE mask-mult frees an st ~1.3us after it is written.  ps_b
    # at 4 gives the pv/rb/proj/qproj rotation slack (4+2*2 = 8 PSUM banks).
    ps_a = ctx.enter_context(tc.tile_pool(name="ps_a", bufs=2, space="PSUM"))
    ps_b = ctx.enter_context(tc.tile_pool(name="ps_b", bufs=4, space="PSUM"))

    wo = []
    for ghc in range(d.NF):
        t = wo_p.tile([128, d.D], BF16, tag="wo", name=f"wo{ghc}")
        nc.sync.dma_start(out=t, in_=wo_d[ts(ghc, 128), :])
        wo.append(t)

    avt_all = {}
    p_tiles = {}
    pv_tiles = {}
    rc_tiles = {}

    # Route pattern per j: rebalances the PSUM-drain / mask-mult / exp
    # stream across DVE and ACT (gpsimd measured pathologically slow for
    # these ops on HW: 3.8x regression -- keep it idle).
    #   C : DVE stt drain+mask -> ACT exp                      (exact)
    #   P : ACT copy-drain(xA) -> DVE mask-mult -> DVE schraudolph
    #   E : DVE stt drain+mask(xA) -> f16 -> DVE schraudolph  (ACT-free)
    ROUTE_PAT = ["C", "P", "P", "C", "P", "E", "E", "P"]

    def emit_st(qc, h, j):
        """One j-pair of scores for head h; route ops drain PSUM to a bf16
        P tile in SBUF."""
        fti = h // 2
        po = (h % 2) * 64
        q_rhs = qt[fti][po:po + 64, ts(qc, d.QC)]
        st = ps_a.tile([128, 2 * d.QC], F32, tag="psa",
                       name=f"st{qc}_{h}_{j}")
        for half in range(2):
            kti = 2 * j + half
            k_lhs = kt[fti][po:po + 64, ts(kti, 128)]
            nc.tensor.matmul(
                st[:, half * d.QC:(half + 1) * d.QC],
                k_lhs, q_rhs, start=True, stop=True,
            )
        mk = mk_tiles[(qc, j)]
        p_t = pb_p.tile([128, 2 * d.QC], BF16, tag="pb",
                        name=f"p{qc}_{h}_{j}")
        route = ROUTE_PAT[j % 8]
        if route == "C":
            msm = stg_p.tile([128, 2 * d.QC], F32, tag="stg",
                             name=f"m{qc}_{h}_{j}")
            nc.vector.tensor_mul(msm, st, mk)
            nc.scalar.activation(p_t, msm, AF.Exp, scale=d.SCALE)
        elif route == "E":
            msm = stg16_p.tile([128, 2 * d.QC], F16, tag="stg16",
                              name=f"m{qc}_{h}_{j}")
            nc.vector.scalar_tensor_tensor(msm, st, d.SCALE * EXP_A, mk,
                                           ALU.mult, ALU.mult)
            nc.vector.tensor_scalar(p_t.bitcast(I16), msm, EXP_B, None,
                                    ALU.add)
        else:  # P: ACT drains PSUM with the xA scale folded in
            m2 = stg16_p.tile([128, 2 * d.QC], F16, tag="stg16",
                              name=f"m2{qc}_{h}_{j}")
            nc.scalar.activation(m2, st, AF.Copy, scale=d.SCALE * EXP_A)
            mm = stg16_p.tile([128, 2 * d.QC], F16, tag="stg16",
                              name=f"mm{qc}_{h}_{j}")
            nc.vector.tensor_mul(mm, m2, mk)
            nc.vector.tensor_scalar(p_t.bitcast(I16), mm, EXP_B, None,
                                    ALU.add)
        p_tiles[(qc, h, j)] = p_t

    def emit_pv(qc, h, j):
        """One j-pair of PV accumulation for head h (P tile ready: one head
        behind the st stream)."""
        if j == 0:
            pv_tiles[(qc, h)] = ps_b.tile([65, d.QC], F32, tag="psb",
                                          name=f"pv{qc}_{h}")
        pv_ps = pv_tiles[(qc, h)]
        p_t = p_tiles.pop((qc, h, j))
        for half in range(2):
            kti = 2 * j + half
            v_lhs = vx[kti][:, h * 65:(h + 1) * 65]
            nc.tensor.matmul(
                pv_ps, v_lhs, p_t[:, half * d.QC:(half + 1) * d.QC],
                start=(kti == 0), stop=(kti == d.NKT - 1),
            )
        if j == d.NKT // 2 - 1:
            # ln(den) on ACT (DVE is the attention bottleneck; keep it lean)
            rc = rows3.tile([1, d.QC], F32R, tag="rows3", name=f"rc{qc}_{h}")
            nc.scalar.activation(rc, pv_ps[64:65, :], AF.Ln)
            rc_tiles[(qc, h)] = rc

    def emit_norm(qc, h):
        """avt = OT_u[0:64] * exp(-bcast(ln den)); frees the pv PSUM tile."""
        fti = h // 2
        po = (h % 2) * 64
        pv_ps = pv_tiles.pop((qc, h))
        rb = ps_b.tile([64, d.QC], F32, tag="psb", name=f"rb{qc}_{h}")
        nc.tensor.matmul(rb, ones_1x64_r, rc_tiles.pop((qc, h)),
                         start=True, stop=True)
        rbe = rbe_p.tile([64, d.QC], F32, tag="rbe", name=f"rbe{qc}_{h}")
        nc.scalar.activation(rbe, rb, AF.Exp, scale=-1.0)
        if (qc, fti) not in avt_all:
            avt_all[(qc, fti)] = avt_p.tile([128, d.QC], BF16, tag="avt",
                                            name=f"avt{qc}_{fti}")
        nc.vector.tensor_mul(avt_all[(qc, fti)][po:po + 64, :],
                             pv_ps[0:64, :], rbe)

    def emit_proj(qc):
        avt = [avt_all[(qc, ft)] for ft in range(d.NF)]
        dcw = min(512, d.D)
        for tt in range(d.QC // 128):
            for dc2 in range(d.D // dcw):
                pp = ps_b.tile([128, dcw], F32, tag="psb",
                               name=f"pp{qc}_{tt}_{dc2}")
                for ghc in range(d.NF):
                    nc.tensor.matmul(
                        pp, avt[ghc][:, ts(tt, 128)], wo[ghc][:, ts(dc2, dcw)],
                        start=(ghc == 0), stop=(ghc == d.NF - 1),
                    )
                ob = ob_p.tile([128, dcw], F32, tag="ob",
                               name=f"ob{qc}_{tt}_{dc2}")
                nc.scalar.copy(ob, pp)
                nc.sync.dma_start(
                    out=out_d[qc * d.QC + tt * 128: qc * d.QC + (tt + 1) * 128,
                              ts(dc2, dcw)],
                    in_=ob,
                )

    # Software-pipelined schedule: PE never waits on a route op, because
    # pv-loop(h-1) is emitted after st-loop(h) (one full head of slack) and
    # the normalize of h-2 trails one more head.  The output projection of
    # the previous q-chunk slots in near the start of the next one.
    for qc in range(d.NQC):
        if qc + 1 < d.NQC:           # prefetch next q-chunk's mask tiles
            for j in range(d.NKT // 2):
                load_mask(qc + 1, j)
        for h in range(d.HL):
            for j in range(d.NKT // 2):
                emit_st(qc, h, j)
                if h >= 1:
                    emit_pv(qc, h - 1, j)
                # normalize of h-2 mid-block: its ln(den) (queued at the
                # tail of the previous block's ACT work) has time to land
                if j == min(2, d.NKT // 2 - 1) and h >= 2:
                    emit_norm(qc, h - 2)
            if h == 1 and qc > 0:
                emit_proj(qc - 1)
            if h == min(3, d.HL - 1) and qc + 1 < d.NQC:
                emit_qproj(qc + 1)   # next q-chunk's Q, in PE slack
        emit_norm(qc, d.HL - 2)
        for j in range(d.NKT // 2):
            emit_pv(qc, d.HL - 1, j)
        emit_norm(qc, d.HL - 1)
    emit_proj(d.NQC - 1)


# =========================================================
# Host-side wrapper
# =========================================================
_B, _T, _D, _H, _DH = 4, 2048, 1024, 16, 64
_NCORES = 8
_CACHE = {}


def _built():
    if "nc" not in _CACHE:
        nc = build_bass(Dims())
        _split_multi_waits(nc)   # HW-compile path only; CoreSim rejects it
        _CACHE["nc"] = nc
    return _CACHE["nc"]


def _aug_w(W, gamma, beta):
    """[D+2, F] = [gamma-scaled W; gamma@W; beta@W] (bf16)."""
    Wg = gamma[:, None] * W
    a = (gamma @ W)[None, :]
    b = (beta @ W)[None, :]
    return np.ascontiguousarray(
        np.concatenate([Wg, a, b], axis=0)).astype(NPBF16)


def _aug_wv(W, gamma, beta, HL=8, DH=64):
    """V weights with an interleaved ones column per head: [D+2, HL*65].
    Column h*65+64 is e_{D+1} so the projection of [xs; c; 1] yields 1.0."""
    D = W.shape[0]
    base = np.concatenate(
        [gamma[:, None] * W, (gamma @ W)[None, :], (beta @ W)[None, :]], axis=0
    )  # [D+2, HL*DH]
    out = np.zeros((D + 2, HL * 65), np.float32)
    for h in range(HL):
        out[:, h * 65:h * 65 + DH] = base[:, h * DH:(h + 1) * DH]
        out[D + 1, h * 65 + DH] = 1.0
    return np.ascontiguousarray(out).astype(NPBF16)


def core_input_map(x_bT, maskT, gamma, beta, Wq_sl, Wk_sl, Wv_sl, Wo_sl,
                   HL=8, DH=64):
    """Per-core parameter dict from fp32 numpy slices (x_bT is [D, T])."""
    return {
        "xT": np.ascontiguousarray(x_bT).astype(NPBF16),
        "maskT": np.ascontiguousarray(maskT).astype(NPBF16),
        "Wq": _aug_w(Wq_sl, gamma, beta),
        "Wk": _aug_w(Wk_sl, gamma, beta),
        "Wv": _aug_wv(Wv_sl, gamma, beta, HL=HL, DH=DH),
        "Wo": np.ascontiguousarray(Wo_sl).astype(NPBF16),
    }


def kernel(x, attn_mask, gamma, beta, Wq, Wk, Wv, Wo, bo):
    x = np.asarray(x, np.float32)
    attn_mask = np.asarray(attn_mask, np.float32)
    gamma = np.asarray(gamma, np.float32)
    beta = np.asarray(beta, np.float32)
    Wq = np.asarray(Wq, np.float32)
    Wk = np.asarray(Wk, np.float32)
    Wv = np.asarray(Wv, np.float32)
    Wo = np.asarray(Wo, np.float32)
    bo = np.asarray(bo, np.float32)

    maskT = np.ascontiguousarray(attn_mask.T)
    F = _D // 2  # 512 features per core

    in_maps = []
    for c in range(_NCORES):
        b, hh = divmod(c, 2)
        sl = slice(hh * F, (hh + 1) * F)
        in_maps.append(core_input_map(
            x[b].T, maskT, gamma, beta,
            Wq[:, sl], Wk[:, sl], Wv[:, sl], Wo[sl, :]))

    from concourse.bass_utils import run_bass_kernel_spmd

    res = run_bass_kernel_spmd(_built(), in_maps, list(range(_NCORES))).results
    out = np.empty((_B, _T, _D), np.float32)
    for b in range(_B):
        out[b] = res[2 * b]["out_p"] + res[2 * b + 1]["out_p"] + bo
    return out

